# revision 35
# baseline (speedup 1.0000x reference)
"""MoE routing kernel for Trainium2 (8 NeuronCores, data-parallel over batch).

Problem: x[B=8,S=2048,D=1024] f32; gate Wg[E=4,D]+bg; experts We[E,D,D]+be.
  gate = x @ Wg.T + bg; top1 = argmax(gate); weights[b,e] = count_e(top1[b])/S
  out[b] = sum_e weights[b,e] * relu(x[b] @ We[e].T + be[e])

The wall-clock of a warm call is dominated by the axon tunnel (~35MB/s each
way) and per-call jit re-tracing, so the layout is chosen to minimize wire
bytes and per-call host work:

  - gate/argmax/weights ([B,E], 128 bytes) are computed on the host in f32
    (137 MFLOP, ~15ms) — exact argmax, no gate matmuls on device.
  - x ships as fp16 [B*S,D] (34MB). The device expert matmuls run in fp16
    (PE: 1 cyc/row, same as bf16, 3 more mantissa bits than the bf16 the
    previous version used).
  - expert weights ship PRE-TRANSPOSED (We[e].T, contraction dim major) in
    fp16 and are cached device-resident across calls, keyed by md5 of the
    f32 bytes. No per-call weight traffic.
  - out comes back as fp16 [B*S,D] (34MB) and is cast to f32 on the host.
  - ONE jitted executable (module-global cache) — no per-call retrace, no
    donation so the dummy zero-out operand is uploaded exactly once.

Per-core device kernel: core c owns batch element c. Load x, PE-transpose
to put the contraction dim on partitions, then for each of the 4 experts
accumulate K=1024 in 8 PSUM chunks; epilogue relu(w_e * y) on ScalarE
(w_e >= 0 folds into the activation scale) + DVE add tree; fp16 out.
"""

import hashlib
from concurrent.futures import ThreadPoolExecutor

import numpy as np

import concourse.bass as bass
import concourse.tile as tile
from concourse import mybir
from concourse.masks import make_identity
from concourse.vector_clock import ScopedClock, VectorClock

F32 = mybir.dt.float32
F16 = mybir.dt.float16
RELU = mybir.ActivationFunctionType.Relu

B, S, D, E = 8, 2048, 1024, 4
P = 128
NK = D // P   # 8 contraction chunks
NC = 512      # matmul moving free dim (one PSUM bank of f32)
ND = D // NC  # 2 dout chunks
CH = int(__import__("os").environ.get("KMOE_CH", "2"))
# token-chunks per call: pipelines H2D(c+1) with exec/D2H(c)
SC = S // CH  # tokens per chunk per core


def _apply_tile_drain_patch():
    """The walrus build in this container only encodes one sync-wait on a
    CTRL instruction; Tile's kernel-tail drain attaches one wait per active
    proc to a single InstDrain and fails codegen. Split it into one drain
    per proc instead."""
    if getattr(tile.TileContext, "_moe_drain_patch", False):
        return
    tile.TileContext._moe_drain_patch = True

    def _drain_and_barrier(self, tick_clock, wait_clock):
        gc = tick_clock.global_clock
        scopes = [(None, gc)] if isinstance(gc, VectorClock) else gc.items()
        n_emitted = 0
        for scope, vc in scopes:
            n = len(vc)
            for proc in range(n):
                t = vc[proc]
                if t > 0:
                    single = VectorClock([t if i == proc else 0 for i in range(n)])
                    d = self.nc.sync.drain()
                    wait_clock.add_sem_waits(d.ins, ScopedClock({scope: single}))
                    n_emitted += 1
        if n_emitted == 0:
            self.nc.sync.drain()
        self.nc.all_engine_barrier()
        popped = self.nc._tile_sem_poison_stack.pop()
        assert popped is self._sem_poison
        self.nc.clear_and_free_semaphores(list(self.sems.allocated().values()))
        self.nc.all_engine_barrier()

    tile.TileContext._drain_and_barrier = _drain_and_barrier


_apply_tile_drain_patch()


def _split_sync_waits(nc: bass.Bass, limit: int = 1):
    """This container's walrus encodes at most one sync-wait per instruction.
    Hoist excess waits onto same-engine NoOps emitted immediately before the
    instruction — the engine stream blocks on each in turn, which is
    semantically identical to waiting on all of them at once."""
    ctr = 0
    for f in nc.m.functions:
        for bb in f.blocks:
            insts = list(bb.instructions)
            out = []
            changed = False
            for ins in insts:
                si = ins.sync_info
                waits = list(si.on_wait) if si is not None else []
                if len(waits) > limit:
                    changed = True
                    for w in waits[:-limit]:
                        ctr += 1
                        nop = mybir.InstNoOp(name=f"wsplit-{ctr}", ins=[], outs=[])
                        nop.engine = ins.engine
                        nop.sync_info = mybir.SyncInfo(on_wait=[w], on_update=[])
                        out.append(nop)
                    ins.sync_info = mybir.SyncInfo(
                        on_wait=waits[-limit:], on_update=list(si.on_update)
                    )
                out.append(ins)
            if changed:
                bb.instructions = out


QSCALE = 126.5  # quant range; < 127 so f32 rounding can't hit int8 saturation


def build_kernel(use_be: bool, s_chunk: int = SC) -> bass.Bass:
    NS = s_chunk // P  # s-tiles in this chunk

    nc = bass.Bass()
    # x arrives as per-token symmetric int8 with NO scale: row scaling is
    # positively homogeneous through matmul/relu/w_e and cancels in the
    # output quantizer, so the host folds rowmax_x/127 into the dequant.
    x_d = nc.dram_tensor("xq", [s_chunk, D], mybir.dt.int8, kind="ExternalInput")
    w_d = nc.dram_tensor("wexp", [E], F32, kind="ExternalInput")
    weT_d = nc.dram_tensor("weT", [E, D, D], F16, kind="ExternalInput")
    be_d = nc.dram_tensor("be16", [E, D], F16, kind="ExternalInput")
    # int8 output + per-token absmax: host reconstructs out = q * rowmax/QSCALE
    out_d = nc.dram_tensor("outq", [s_chunk, D], mybir.dt.int8,
                           kind="ExternalOutput")
    osc_d = nc.dram_tensor("osc", [P, NS], F32, kind="ExternalOutput")

    with tile.TileContext(nc) as tc:
        const = tc.alloc_tile_pool(name="const", bufs=1)
        big = tc.alloc_tile_pool(name="big", bufs=1)
        stage = tc.alloc_tile_pool(name="stage", bufs=4)
        psum_tr = tc.alloc_tile_pool(name="psum_tr", bufs=3, space="PSUM")

        ident = const.tile([P, P], F16)
        make_identity(nc, ident)

        # per-expert scalar weights broadcast to all partitions (stride-0 DMA)
        w_bc = const.tile([P, E], F32)
        nc.gpsimd.dma_start(
            out=w_bc, in_=bass.AP(tensor=w_d, offset=0, ap=[[0, P], [1, E]])
        )
        oscales = const.tile([P, NS], F32)
        if use_be:
            be_sb = const.tile([E, D], F16)
            nc.sync.dma_start(out=be_sb, in_=be_d[:, :])
            ones_row = const.tile([1, P], F16)
            nc.vector.memset(ones_row, 1.0)

        # persistent transposed x and natural-layout (pre-transposed on host)
        # expert weights
        xT = big.tile([P, NK, NS, P], F16)     # 2*NS KB/partition
        weT_sb = big.tile([P, E, NK, D], F16)  # 64 KB/partition

        for e in range(E):
            for k in range(NK):
                nc.sync.dma_start(
                    out=weT_sb[:, e, k, :], in_=weT_d[e, k * P : (k + 1) * P, :]
                )

        for st in range(NS):
            x_q = stage.tile([P, D], mybir.dt.int8, tag="stq")
            nc.sync.dma_start(out=x_q, in_=x_d[st * P : (st + 1) * P, :])
            # int8 -> fp16 (integers <= 127 are exact in fp16)
            x_nat = stage.tile([P, D], F16, tag="stg")
            nc.scalar.activation(
                x_nat, x_q, mybir.ActivationFunctionType.Copy
            )
            ptr = psum_tr.tile([P, NK, P], F16, tag="ptr")
            for k in range(NK):
                nc.tensor.matmul(
                    ptr[:, k, :],
                    x_nat[:, k * P : (k + 1) * P],
                    ident,
                    is_transpose=True,
                    start=True,
                    stop=True,
                )
            nc.scalar.copy(xT[:, :, st, :], ptr)

        psum_tr.release()

        # --- expert matmuls + fused epilogue ---
        psum_main = tc.alloc_tile_pool(name="psum_main", bufs=4, space="PSUM")
        relu_p = tc.alloc_tile_pool(name="relu_p", bufs=6)
        acc_p = tc.alloc_tile_pool(name="acc_p", bufs=4)
        out_p = tc.alloc_tile_pool(name="out_p", bufs=3)

        for st in range(NS):
            accs = []
            for half in range(2):
                pts = [
                    psum_main.tile([P, D], F32, tag="pm", name=f"pm{e2}")
                    for e2 in range(2)
                ]
                if use_be:
                    for e2, pt in enumerate(pts):
                        e = half * 2 + e2
                        for c in range(ND):
                            nc.tensor.matmul(
                                pt[:, c * NC : (c + 1) * NC],
                                ones_row,
                                be_sb[e : e + 1, c * NC : (c + 1) * NC],
                                start=True, stop=False,
                            )
                for k in range(NK):
                    lhs = xT[:, k, st, :]
                    for e2, pt in enumerate(pts):
                        e = half * 2 + e2
                        for c in range(ND):
                            nc.tensor.matmul(
                                pt[:, c * NC : (c + 1) * NC],
                                lhs,
                                weT_sb[:, e, k, c * NC : (c + 1) * NC],
                                start=(k == 0 and not use_be),
                                stop=(k == NK - 1),
                            )
                trs = []
                for e2, pt in enumerate(pts):
                    e = half * 2 + e2
                    tr = relu_p.tile([P, D], F16, tag="tr")
                    nc.scalar.activation(tr, pt, RELU, scale=w_bc[:, e : e + 1])
                    trs.append(tr)
                acc = acc_p.tile([P, D], F32, tag="acc")
                nc.vector.tensor_add(acc, trs[0], trs[1])
                accs.append(acc)
            o = out_p.tile([P, D], F32, tag="o")
            nc.vector.tensor_add(o, accs[0], accs[1])
            # per-token int8 quantization: q = o * QSCALE/rowmax(o); o >= 0
            # (weights and relu are nonneg) so max == absmax. The host
            # dequantizes with 1/qsc, so reciprocal error cancels exactly.
            rmax = out_p.tile([P, 1], F32, tag="rmax")
            nc.vector.tensor_reduce(
                rmax, o, axis=mybir.AxisListType.X, op=mybir.AluOpType.max
            )
            recip = out_p.tile([P, 1], F32, tag="recip")
            nc.vector.reciprocal(recip, rmax)
            qsc = oscales[:, st : st + 1]
            nc.scalar.mul(qsc, recip, QSCALE)
            q = out_p.tile([P, D], mybir.dt.int8, tag="q")
            nc.scalar.activation(
                q, o, mybir.ActivationFunctionType.Copy, scale=qsc[:, 0:1]
            )
            nc.sync.dma_start(out=out_d[st * P : (st + 1) * P, :], in_=q)
        nc.sync.dma_start(out=osc_d[:, :], in_=oscales)

        out_p.release()
        acc_p.release()
        relu_p.release()
        psum_main.release()
        stage.release()
        big.release()
        const.release()

    _split_sync_waits(nc)
    return nc


# ---------------------------------------------------------------------------
# host orchestration: cached jitted executable + device-resident weights
# ---------------------------------------------------------------------------

_STATE: dict = {}
_FETCH_POOL = ThreadPoolExecutor(2)
_WORK_POOL = ThreadPoolExecutor(4)


def _quant_chunk(xf):
    """Per-token symmetric int8 over [B, SC, D], threaded over sub-blocks."""
    nb, sc, d = xf.shape
    rowmax = np.empty((nb, sc), np.float32)
    xq = np.empty((nb, sc, d), np.int8)

    def do(b):
        xb = xf[b]
        rm = np.abs(xb).max(axis=1)
        rowmax[b] = rm
        inv = np.float32(127.0) / np.maximum(rm, np.float32(1e-30))
        np.rint(xb * inv[:, None], casting="unsafe", out=xq[b])

    list(_WORK_POOL.map(do, range(nb)))
    return xq, rowmax


def _get_exec(use_be: bool):
    """Build (once) the Bass module and a reusable jitted SPMD callable."""
    key = ("exec", use_be)
    if key in _STATE:
        return _STATE[key]

    import jax
    from jax.sharding import Mesh, PartitionSpec, NamedSharding
    from jax.experimental.shard_map import shard_map
    from concourse import bass2jax

    nc = build_kernel(use_be, SC)
    bass2jax.install_neuronx_cc_hook()

    partition_name = (
        nc.partition_id_tensor.name if nc.partition_id_tensor else None
    )
    in_names, out_names, out_avals = [], [], []
    for alloc in nc.m.functions[0].allocations:
        if not isinstance(alloc, mybir.MemoryLocationSet):
            continue
        name = alloc.memorylocations[0].name
        if alloc.kind == "ExternalInput":
            if name != partition_name:
                in_names.append(name)
        elif alloc.kind == "ExternalOutput":
            out_names.append(name)
            out_avals.append(
                jax.core.ShapedArray(
                    tuple(alloc.tensor_shape), mybir.dt.np(alloc.dtype)
                )
            )
    n_params = len(in_names)
    all_names = in_names + out_names
    if partition_name is not None:
        all_names = all_names + [partition_name]

    def _body(*args):
        operands = list(args)
        if partition_name is not None:
            operands.append(bass2jax.partition_id_tensor())
        outs = bass2jax._bass_exec_p.bind(
            *operands,
            out_avals=tuple(out_avals),
            in_names=tuple(all_names),
            out_names=tuple(out_names),
            lowering_input_output_aliases=(),
            sim_require_finite=True,
            sim_require_nnan=True,
            nc=nc,
        )
        return tuple(outs)

    devices = jax.devices()[:B]
    mesh = Mesh(np.asarray(devices), ("core",))
    in_specs = (PartitionSpec("core"),) * (n_params + len(out_names))
    out_specs = (PartitionSpec("core"),) * len(out_names)
    # no donation: the dummy zero 'out' operand buffer stays alive and is
    # reused every call (the NEFF writes the custom-call result buffer, it
    # never reads this operand)
    jfn = jax.jit(
        shard_map(
            _body, mesh=mesh, in_specs=in_specs, out_specs=out_specs,
            check_rep=False,
        ),
        keep_unused=True,
    )
    sh = NamedSharding(mesh, PartitionSpec("core"))
    zeros_dev = [
        jax.device_put(
            np.zeros((B * av.shape[0], *av.shape[1:]), av.dtype), sh
        )
        for av in out_avals
    ]
    st = {
        "jfn": jfn,
        "sh": sh,
        "in_names": in_names,
        "zeros": zeros_dev,
        "device_put": jax.device_put,
    }
    _STATE[key] = st
    return st


def _get_weights_dev(st, We, be):
    """Device-resident fp16 pre-transposed expert weights, cached by content."""
    h = hashlib.md5(We.data).hexdigest() + hashlib.md5(be.data).hexdigest()
    key = ("weights", h)
    if key in _STATE:
        return _STATE[key]
    weT16 = np.ascontiguousarray(We.transpose(0, 2, 1)).astype(np.float16)
    weT_cat = np.tile(weT16, (B, 1, 1))              # [B*E, D, D]
    be_cat = np.tile(be.astype(np.float16), (B, 1))  # [B*E, D]
    weT_dev = st["device_put"](weT_cat, st["sh"])
    be_dev = st["device_put"](be_cat, st["sh"])
    _STATE[key] = (weT_dev, be_dev)
    return _STATE[key]


def kernel(x, Wg, bg, We, be):
    x = np.ascontiguousarray(np.asarray(x, dtype=np.float32))
    Wg = np.ascontiguousarray(np.asarray(Wg, dtype=np.float32))
    bg = np.ascontiguousarray(np.asarray(bg, dtype=np.float32))
    We = np.ascontiguousarray(np.asarray(We, dtype=np.float32))
    be = np.ascontiguousarray(np.asarray(be, dtype=np.float32))
    assert x.shape == (B, S, D) and Wg.shape == (E, D)
    assert We.shape == (E, D, D) and bg.shape == (E,) and be.shape == (E, D)

    st = _get_exec(use_be=bool(np.any(be)))
    put, jfn, sh = st["device_put"], st["jfn"], st["sh"]

    xv = x.reshape(B, CH, SC, D)
    NS = SC // P
    wfut = _FETCH_POOL.submit(_get_weights_dev, st, We, be)

    # quantize + enqueue all chunk uploads first — the wire starts moving
    # while the gate/argmax and the weight-cache hash run on the host.
    xq_devs, rowmaxes = [], []
    for c in range(CH):
        # scale-free per-token int8; rowmax/127 folds into the host dequant
        xq, rowmax = _quant_chunk(xv[:, c])
        xq_devs.append(put(xq.reshape(B * SC, D), sh))
        rowmaxes.append(rowmax)

    # --- routing on the host (exact f32 argmax; 137 MFLOP ~ 15ms) ---
    gate = x @ Wg.T + bg                      # [B,S,E]
    top1 = np.argmax(gate, axis=-1)           # [B,S]
    counts = np.zeros((B, E), np.float32)
    for e in range(E):
        counts[:, e] = (top1 == e).sum(axis=1)
    w_all = (counts / S).reshape(B * E)       # per-core [E] after sharding

    weT_dev, be_dev = wfut.result()
    w_dev = st["device_put"](w_all, st["sh"])

    # launch all chunks (async); a worker thread blocks on each chunk's D2H
    # so downloads overlap the remaining uploads on the tunnel.
    futs = []
    for c in range(CH):
        by_name = {
            "xq": xq_devs[c],
            "wexp": w_dev,
            "weT": weT_dev,
            "be16": be_dev,
        }
        args = [by_name[n] for n in st["in_names"]] + st["zeros"]
        q_c, sc_c = jfn(*args)

        def _fetch(q=q_c, s=sc_c):
            for a in (q, s):
                try:
                    a.copy_to_host_async()
                except Exception:
                    pass
            return np.asarray(q), np.asarray(s)

        futs.append(_FETCH_POOL.submit(_fetch))

    res = np.empty((B, S, D), np.float32)
    rv = res.reshape(B, CH, SC, D)
    for c in range(CH):
        q, osc = futs[c].result()
        # osc is [B*P, NS] holding qsc = QSCALE/rowmax(o'); o' = o*127/rowmax_x
        # per token t = st*P + p  ->  out = q/qsc * rowmax_x/127
        oscr = osc.reshape(B, P, NS).transpose(0, 2, 1).reshape(B, SC)
        scale = rowmaxes[c] / (np.float32(127.0) * oscr)
        np.multiply(
            q.reshape(B, SC, D), scale[:, :, None].astype(np.float32),
            out=rv[:, c], casting="unsafe",
        )
    return res
